# revision 21
# baseline (speedup 1.0000x reference)
"""EquiAttention Trainium2 kernel (v2).

Data-parallel over batch: 64 batches -> 8 per core, seq N=1024.

Math (folded form validated exact in float64, see _host_weights):
  softmax scores fold to a 128-dim contraction plus a per-key bias:
    S[q,m] ~ q128_q . (BD^T q128)_m + c2.s_m   (mod per-query const)
  with q128 = [normalized vecs (64) | scalars (64)].
  V = (exp(Wv)-proj vecs); denominator via an all-ones 65th V column.

Device structure per batch (all layouts feature-major, prepped on host):
  - qT [128,N] f32r arrives pre-normalized/transposed from host (the
    O(B*N) pointwise Lorentz normalization + layout transform are host
    prep; all O(B*N^2) attention math runs on device).
  - kT = BD^T qT: one fp32r matmul pair (fp32r streams 1 cycle/row at
    moving>=256 vs fp32's 4 -- measured 230ns vs 860ns per 512-wide).
  - Scores are computed TRANSPOSED, per key-chunk mc:
      S^T[mc] [128m, 1024q] = matmul(kt chunk stationary, qT moving)
    so exp(S^T) IS P^T directly -- no DMA/PE transposes of P at all.
  - exp: one 1024-wide ACT instruction per chunk reading the 2-bank
    PSUM tile, per-partition bias = c2.s_m - 20 (constant shift instead
    of a per-row max: removes the DVE row-max reduce and its serial
    dependency). P stored bf16 (dynamic range to 3e38).
  - PV: accT[65,1024] += Vaug[mc]^T @ P^T[mc] (bf16), denominator from
    the ones column.  accT is DMA'd out raw as [65,N]; the final
    divide + [65,N]->[N,64] transpose happen on host (0.05% of FLOPs).
  - Outlier rows (rowmax > ~85 after the -20 shift overflow fp32/bf16;
    18 of 65536 rows in this distribution, winner-take-all structure)
    are detected on host via den >= e^65 or nonfinite and recomputed
    exactly in numpy (~0.3 MFLOP/row).
"""

import numpy as np

B, N = 64, 1024
NCORES = 8
BL = B // NCORES          # batches per core
NB = N // 128             # 128-row key chunks
SCALE = 1.0 / np.sqrt(192.0)
CSHIFT = 20.0             # constant softmax shift (rowmax p99.9 = 48)
FLAG_LOGDEN = 65.0        # host-recompute rows with log(den) above this

_CACHE = {}


def _build_program():
    import concourse.bacc as bacc
    import concourse.tile as tile
    from concourse import mybir

    f32 = mybir.dt.float32
    f32r = mybir.dt.float32r
    f16 = mybir.dt.float16

    nc = bacc.Bacc("TRN2", target_bir_lowering=False,
                   debug=False, num_devices=NCORES)

    bf16 = mybir.dt.bfloat16
    f32 = mybir.dt.float32
    aps = {
        "qT": nc.dram_tensor("qT", [BL, 128, N], f32r,
                             kind="ExternalInput").ap(),
        "BD": nc.dram_tensor("BD", [128, 128], f32r,
                             kind="ExternalInput").ap(),
        "vaug": nc.dram_tensor("vaug", [BL, 128, NB, 65], bf16,
                               kind="ExternalInput").ap(),
        "bcol": nc.dram_tensor("bcol", [BL, 128, NB], f32,
                               kind="ExternalInput").ap(),
        "out": nc.dram_tensor("out", [BL, 65, N], f32,
                              kind="ExternalOutput").ap(),
    }

    with tile.TileContext(nc) as tc:
        _emit(tc, aps)

    nc.compile()
    return nc


def _emit(tc, aps):
    from contextlib import ExitStack
    from concourse import mybir

    nc = tc.nc
    f32 = mybir.dt.float32
    f32r = mybir.dt.float32r
    f16 = mybir.dt.float16
    bf16 = mybir.dt.bfloat16
    PS = "PSUM"
    Act = mybir.ActivationFunctionType

    qT_d, bd_d, out_d = aps["qT"], aps["BD"], aps["out"]
    vaug_d, bcol_d = aps["vaug"], aps["bcol"]

    with ExitStack() as ctx:
        singles = ctx.enter_context(tc.tile_pool(name="singles", bufs=1))
        per = ctx.enter_context(tc.tile_pool(name="per", bufs=3))
        pP = ctx.enter_context(tc.tile_pool(name="pP", bufs=2))
        # PSUM: 2 x [128,1024] (2 banks each) rotating for scores,
        # 1 x [128,1024] dedicated to embed (kT then vaug, sequential),
        # 1 x [65,1024] for the PV accumulator -> 8 banks exactly.
        psS = ctx.enter_context(tc.tile_pool(name="psS", bufs=2, space=PS))
        psE = ctx.enter_context(tc.tile_pool(name="psE", bufs=1, space=PS))
        psAcc = ctx.enter_context(tc.tile_pool(name="psAcc", bufs=1, space=PS))

        bd = singles.tile([128, 128], f32r)
        nc.sync.dma_start(out=bd[:], in_=bd_d[:, :])

        # Preload the Exp ACT table + warm the PE p-state while the first
        # input DMAs are in flight (HAM needs sustained activity for the
        # 2.4GHz clock; cold matmuls run at half speed otherwise).
        wtile = singles.tile([128, 512], f16)
        nc.gpsimd.memset(wtile[:], 0.0)
        pre = singles.tile([128, 1], f16)
        nc.gpsimd.memset(pre[:], 0.0)
        pre2 = singles.tile([128, 1], bf16)
        nc.scalar.activation(out=pre2[:], in_=pre[:], func=Act.Exp, scale=1.0)
        pw = psE.tile([128, N], f32, tag="E")
        for _ in range(12):
            nc.tensor.matmul(pw[:, 0:512], wtile[:, 0:128], wtile[:],
                             start=True, stop=True)

        def fetch_qT(b):
            qT = per.tile([128, N], f32r, tag="qT")
            nc.sync.dma_start(out=qT[:], in_=qT_d[b])
            vaug = per.tile([128, NB, 65], bf16, tag="vaug")
            nc.sync.dma_start(out=vaug[:], in_=vaug_d[b])
            bcol = per.tile([128, NB], f32, tag="bcol")
            nc.sync.dma_start(out=bcol[:], in_=bcol_d[b])
            return qT, vaug, bcol

        def embed_steps(b, fetched):
            """Generator yielding small embed work pieces, to be smeared
            across the exp-slack of the surrounding scores round."""
            qT, vaug, bcol = fetched
            # kT = BD^T qT (fp32r); one matmul half per step
            pk = psE.tile([128, N], f32, tag="E")
            kt = per.tile([128, N], f32r, tag="kt")
            nc.tensor.matmul(pk[:, 0:512], bd[:], qT[:, 0:512],
                             start=True, stop=True)
            yield None
            nc.tensor.matmul(pk[:, 512:1024], bd[:], qT[:, 512:1024],
                             start=True, stop=True)
            nc.vector.tensor_copy(kt[:], pk[:])
            yield (qT, kt, vaug, bcol)

        def pv_chunk(prev, accT, mc):
            # accT[65, q] += Vaug[mc]^T @ P^T[mc] for both halves
            _, pt, vaug = prev
            for hh in range(2):
                cols = slice(hh * 512, (hh + 1) * 512)
                nc.tensor.matmul(accT[:, cols], vaug[:, mc, :],
                                 pt[:, mc, cols],
                                 start=(mc == 0), stop=(mc == NB - 1))

        def pv_drain(b_prev, accT, split=False):
            osb = per.tile([65, N], f32, tag="osb")
            if split:
                nc.vector.tensor_copy(osb[:, 0:512], accT[:, 0:512])
                nc.sync.dma_start(out=out_d[b_prev, :, 0:512],
                                  in_=osb[:, 0:512])
                nc.vector.tensor_copy(osb[:, 512:1024], accT[:, 512:1024])
                nc.sync.dma_start(out=out_d[b_prev, :, 512:1024],
                                  in_=osb[:, 512:1024])
            else:
                nc.vector.tensor_copy(osb[:], accT[:])
                nc.sync.dma_start(out=out_d[b_prev], in_=osb[:])

        def scores_round(b, emb_tiles, prev, emb_gen):
            """Emit scores+exp for batch b, interleaved with the PV of the
            previous batch and the (smeared) embed of the next batch, so
            the PE queue never has a long exp-gated stretch."""
            qT, kt, vaug, bcol = emb_tiles
            pt = pP.tile([128, NB, N], bf16, tag="pt")
            if prev is not None:
                accT = psAcc.tile([65, N], f32, tag="acc")
            else:
                accT = None
            nxt = None
            for mc in range(NB):
                S = psS.tile([128, N], f32, tag="S")
                lhs = kt[:, mc * 128:(mc + 1) * 128]
                nc.tensor.matmul(S[:, 0:512], lhs, qT[:, 0:512],
                                 start=True, stop=True)
                nc.tensor.matmul(S[:, 512:1024], lhs, qT[:, 512:1024],
                                 start=True, stop=True)
                nc.scalar.activation(out=pt[:, mc, :], in_=S[:],
                                     func=Act.Exp,
                                     bias=bcol[:, mc:mc + 1], scale=1.0)
                if prev is not None:
                    pv_chunk(prev, accT, mc)
                if emb_gen is not None and mc >= 1:
                    nxt = next(emb_gen, nxt) or nxt
            if prev is not None:
                pv_drain(prev[0], accT)
            return nxt, (b, pt, vaug)

        prev = None
        qt0 = fetch_qT(0)
        qt_next = fetch_qT(1)
        emb = None
        for emb in embed_steps(0, qt0):
            pass
        for b in range(BL):
            if b + 1 < BL:
                emb_gen = embed_steps(b + 1, qt_next)
                if b + 2 < BL:
                    qt_next = fetch_qT(b + 2)
            else:
                emb_gen = None
            nxt, prev = scores_round(b, emb, prev, emb_gen)
            emb = nxt
        accT = psAcc.tile([65, N], f32, tag="acc")
        for mc in range(NB):
            pv_chunk(prev, accT, mc)
        pv_drain(BL - 1, accT, split=True)


def _host_weights(Wq, Wk, Wv, Wq_s, Wk_s, bq_s):
    """Fold the tiny EquiLinear weights (float64 precompute)."""
    METRIC = np.array([1.0, -1.0, -1.0, -1.0], dtype=np.float64)
    G = Wq.astype(np.float64).T @ Wk.astype(np.float64)            # [16,16]
    BD = np.zeros((128, 128), dtype=np.float64)
    for k in range(4):
        # lhsT[(j',k), (j,k)] = SCALE * METRIC[k] * G[j, j']
        BD[k:64:4, k:64:4] = SCALE * METRIC[k] * G.T
    # lhsT[h, g] = SCALE * H[g, h],  H = Wq_s.T @ Wk_s
    BD[64:, 64:] = SCALE * (Wk_s.astype(np.float64).T @ Wq_s.astype(np.float64))
    E = np.exp(Wv.astype(np.float64))                              # [16,16]
    WvC2 = np.zeros((128, 65), dtype=np.float64)
    for k in range(4):
        # rhs[(j,k), (i,k)] = E[i, j]
        WvC2[k:64:4, k:64:4] = E.T
    WvC2[64:, 64] = SCALE * (Wk_s.astype(np.float64).T @ bq_s.astype(np.float64))
    return BD, WvC2


def _host_prep(vectors, scalars):
    """Lorentz-normalize and build q128^T = [vecs|scalars]^T per batch."""
    METRIC = np.array([1.0, -1.0, -1.0, -1.0], dtype=np.float32)
    v = np.asarray(vectors, dtype=np.float32)
    sq = v * v
    norm = (sq[..., 0] - sq[..., 1] - sq[..., 2] - sq[..., 3])[..., None]
    vecs = v / np.sqrt(np.clip(np.abs(norm), 1e-5, None))
    q128 = np.concatenate(
        [vecs.reshape(B, N, 64), np.asarray(scalars, dtype=np.float32)],
        axis=-1)                                      # [B, N, 128]
    qT = np.ascontiguousarray(q128.transpose(0, 2, 1))  # [B, 128, N]
    return qT, vecs.reshape(B, N, 64)


def _prepare_in_maps(vectors, scalars, Wq, Wq_s, bq_s, Wk, Wk_s, bk_s, Wv):
    import ml_dtypes
    BD, WvC2 = _host_weights(Wq, Wk, Wv, Wq_s, Wk_s, bq_s)
    qT, vecs = _host_prep(vectors, scalars)
    BD32 = np.ascontiguousarray(BD, dtype=np.float32)
    # Vaug (the tiny O(N) V-projection) + exp bias column, host-prepped:
    #   proj[b, m, 0:64] = V, proj[b, m, 64] = c2.s_m
    proj = np.einsum('bdm,dv->bmv', qT, WvC2.astype(np.float32),
                     optimize=True)                    # [B, N, 65]
    vaug = proj.reshape(B, NB, 128, 65).transpose(0, 2, 1, 3).copy()
    vaug[:, :, :, 64] = 1.0
    vaug16 = vaug.astype(ml_dtypes.bfloat16)           # [B, 128, NB, 65]
    bcol = (proj[:, :, 64] - CSHIFT).reshape(B, NB, 128) \
        .transpose(0, 2, 1).astype(np.float32)         # [B, 128, NB]
    in_maps = []
    for c in range(NCORES):
        sl = slice(c * BL, (c + 1) * BL)
        in_maps.append({
            "qT": np.ascontiguousarray(qT[sl]),
            "BD": BD32,
            "vaug": np.ascontiguousarray(vaug16[sl]),
            "bcol": np.ascontiguousarray(bcol[sl]),
        })
    return in_maps, (BD, WvC2, qT, vecs)


def _run(in_maps, **kw):
    from concourse.bass_utils import run_bass_kernel_spmd
    nc = _get_program()
    return run_bass_kernel_spmd(nc, in_maps, list(range(NCORES)), **kw)


def _get_program():
    if "nc" not in _CACHE:
        _CACHE["nc"] = _build_program()
    return _CACHE["nc"]


def _host_patch_row(b, q, BD, WvC2, qT, vecs):
    """Exact fp64 recompute of one (batch, query) output row."""
    q128 = qT[b].astype(np.float64)                   # [128, N]
    kq = BD @ q128[:, q]                              # [128]
    s_col = q128.T @ kq                               # S^T[m, q] = q_m^T BD q_q
    bias = q128[64:, :].T @ WvC2[64:, 64]             # c2 . s_m
    S = s_col + bias
    S -= S.max()
    P = np.exp(S)
    V = vecs[b].astype(np.float64) @ WvC2[0:64, 0:64]  # E-proj [N, 64]
    return (P @ V) / P.sum()


def kernel(vectors, scalars, Wq, Wq_s, bq_s, Wk, Wk_s, bk_s, Wv):
    args = [np.asarray(a, dtype=np.float32) for a in
            (vectors, scalars, Wq, Wq_s, bq_s, Wk, Wk_s, bk_s, Wv)]
    in_maps, host_ctx = _prepare_in_maps(*args)
    res = _run(in_maps)
    acc = np.concatenate([res.results[c]["out"] for c in range(NCORES)],
                         axis=0)                      # [B, 65, N]
    num = acc[:, 0:64, :]
    den = acc[:, 64, :]
    with np.errstate(divide="ignore", invalid="ignore", over="ignore"):
        out = (num / den[:, None, :]).transpose(0, 2, 1)   # [B, N, 64]
    # patch overflow-outlier rows exactly on host
    BD, WvC2, qT, vecs = host_ctx
    with np.errstate(over="ignore", invalid="ignore"):
        bad = ~np.isfinite(den) | (den <= 0) | \
            (np.log(np.maximum(den, 1e-30)) > FLAG_LOGDEN) | \
            ~np.isfinite(out).all(axis=2)
    for b, q in zip(*np.nonzero(bad)):
        out[b, q] = _host_patch_row(b, q, BD, WvC2, qT, vecs)
    return out.reshape(B, N, 16, 4).astype(np.float32)


# revision 22
# speedup vs baseline: 1.1467x; 1.1467x over previous
"""EquiAttention Trainium2 kernel (v2).

Data-parallel over batch: 64 batches -> 8 per core, seq N=1024.

Math (folded form validated exact in float64, see _host_weights):
  softmax scores fold to a 128-dim contraction plus a per-key bias:
    S[q,m] ~ q128_q . (BD^T q128)_m + c2.s_m   (mod per-query const)
  with q128 = [normalized vecs (64) | scalars (64)].
  V = (exp(Wv)-proj vecs); denominator via an all-ones 65th V column.

Device structure per batch (all layouts feature-major, prepped on host):
  - qT [128,N] f32r arrives pre-normalized/transposed from host (the
    O(B*N) pointwise Lorentz normalization + layout transform are host
    prep; all O(B*N^2) attention math runs on device).
  - kT = BD^T qT: one fp32r matmul pair (fp32r streams 1 cycle/row at
    moving>=256 vs fp32's 4 -- measured 230ns vs 860ns per 512-wide).
  - Scores are computed TRANSPOSED, per key-chunk mc:
      S^T[mc] [128m, 1024q] = matmul(kt chunk stationary, qT moving)
    so exp(S^T) IS P^T directly -- no DMA/PE transposes of P at all.
  - exp: one 1024-wide ACT instruction per chunk reading the 2-bank
    PSUM tile, per-partition bias = c2.s_m - 20 (constant shift instead
    of a per-row max: removes the DVE row-max reduce and its serial
    dependency). P stored bf16 (dynamic range to 3e38).
  - PV: accT[65,1024] += Vaug[mc]^T @ P^T[mc] (bf16), denominator from
    the ones column.  accT is DMA'd out raw as [65,N]; the final
    divide + [65,N]->[N,64] transpose happen on host (0.05% of FLOPs).
  - Outlier rows (rowmax > ~85 after the -20 shift overflow fp32/bf16;
    18 of 65536 rows in this distribution, winner-take-all structure)
    are detected on host via den >= e^65 or nonfinite and recomputed
    exactly in numpy (~0.3 MFLOP/row).
"""

import numpy as np

B, N = 64, 1024
NCORES = 8
BL = B // NCORES          # batches per core
NB = N // 128             # 128-row key chunks
SCALE = 1.0 / np.sqrt(192.0)
CSHIFT = 20.0             # constant softmax shift (rowmax p99.9 = 48)
FLAG_LOGDEN = 65.0        # host-recompute rows with log(den) above this

_CACHE = {}


def _build_program():
    import concourse.bacc as bacc
    import concourse.tile as tile
    from concourse import mybir

    f32 = mybir.dt.float32
    f32r = mybir.dt.float32r
    f16 = mybir.dt.float16

    nc = bacc.Bacc("TRN2", target_bir_lowering=False,
                   debug=False, num_devices=NCORES)

    bf16 = mybir.dt.bfloat16
    f32 = mybir.dt.float32
    aps = {
        "qT": nc.dram_tensor("qT", [BL, 128, N], f32r,
                             kind="ExternalInput").ap(),
        "BD": nc.dram_tensor("BD", [128, 128], f32r,
                             kind="ExternalInput").ap(),
        "vaug": nc.dram_tensor("vaug", [BL, 128, NB, 65], bf16,
                               kind="ExternalInput").ap(),
        "bcol": nc.dram_tensor("bcol", [BL, 128, NB], f32,
                               kind="ExternalInput").ap(),
        "out": nc.dram_tensor("out", [BL, 65, N], f32,
                              kind="ExternalOutput").ap(),
    }

    with tile.TileContext(nc) as tc:
        _emit(tc, aps)

    nc.compile()
    return nc


def _emit(tc, aps):
    from contextlib import ExitStack
    from concourse import mybir

    nc = tc.nc
    f32 = mybir.dt.float32
    f32r = mybir.dt.float32r
    f16 = mybir.dt.float16
    bf16 = mybir.dt.bfloat16
    PS = "PSUM"
    Act = mybir.ActivationFunctionType

    qT_d, bd_d, out_d = aps["qT"], aps["BD"], aps["out"]
    vaug_d, bcol_d = aps["vaug"], aps["bcol"]

    with ExitStack() as ctx:
        singles = ctx.enter_context(tc.tile_pool(name="singles", bufs=1))
        per = ctx.enter_context(tc.tile_pool(name="per", bufs=3))
        pP = ctx.enter_context(tc.tile_pool(name="pP", bufs=2))
        # PSUM: 2 x [128,1024] (2 banks each) rotating for scores,
        # 1 x [128,1024] dedicated to embed (kT then vaug, sequential),
        # 1 x [65,1024] for the PV accumulator -> 8 banks exactly.
        psS = ctx.enter_context(tc.tile_pool(name="psS", bufs=3, space=PS))
        psAcc = ctx.enter_context(tc.tile_pool(name="psAcc", bufs=1, space=PS))

        bd = singles.tile([128, 128], f32r)
        nc.gpsimd.dma_start(out=bd[:], in_=bd_d[:, :])

        # Preload the Exp ACT table + warm the PE p-state while the first
        # input DMAs are in flight (HAM needs sustained activity for the
        # 2.4GHz clock; cold matmuls run at half speed otherwise).
        wtile = singles.tile([128, 512], f16)
        nc.gpsimd.memset(wtile[:], 0.0)
        pre = singles.tile([128, 1], f16)
        nc.gpsimd.memset(pre[:], 0.0)
        pre2 = singles.tile([128, 1], bf16)
        nc.scalar.activation(out=pre2[:], in_=pre[:], func=Act.Exp, scale=1.0)
        pw = psS.tile([128, N], f32, tag="S")
        for _ in range(12):
            nc.tensor.matmul(pw[:, 0:512], wtile[:, 0:128], wtile[:],
                             start=True, stop=True)

        def fetch_qT(b):
            qT = per.tile([128, N], f32r, tag="qT")
            nc.gpsimd.dma_start(out=qT[:], in_=qT_d[b])
            vaug = per.tile([128, NB, 65], bf16, tag="vaug")
            nc.gpsimd.dma_start(out=vaug[:], in_=vaug_d[b])
            bcol = per.tile([128, NB], f32, tag="bcol")
            nc.gpsimd.dma_start(out=bcol[:], in_=bcol_d[b])
            return qT, vaug, bcol

        def embed_steps(b, fetched):
            """Generator yielding small embed work pieces, to be smeared
            across the exp-slack of the surrounding scores round."""
            qT, vaug, bcol = fetched
            # kT = BD^T qT (fp32r); one matmul half per step
            pk = psS.tile([128, N], f32, tag="S")
            kt = per.tile([128, N], f32r, tag="kt")
            nc.tensor.matmul(pk[:, 0:512], bd[:], qT[:, 0:512],
                             start=True, stop=True)
            yield None
            nc.tensor.matmul(pk[:, 512:1024], bd[:], qT[:, 512:1024],
                             start=True, stop=True)
            nc.vector.tensor_copy(kt[:], pk[:])
            yield (qT, kt, vaug, bcol)

        def pv_chunk(prev, accT, mc):
            # accT[65, q] += Vaug[mc]^T @ P^T[mc] for both halves
            _, pt, vaug = prev
            for hh in range(2):
                cols = slice(hh * 512, (hh + 1) * 512)
                nc.tensor.matmul(accT[:, cols], vaug[:, mc, :],
                                 pt[:, mc, cols],
                                 start=(mc == 0), stop=(mc == NB - 1))

        def pv_drain(b_prev, accT, split=False):
            osb = per.tile([65, N], f32, tag="osb")
            if split:
                nc.vector.tensor_copy(osb[:, 0:512], accT[:, 0:512])
                nc.sync.dma_start(out=out_d[b_prev, :, 0:512],
                                  in_=osb[:, 0:512])
                nc.vector.tensor_copy(osb[:, 512:1024], accT[:, 512:1024])
                nc.sync.dma_start(out=out_d[b_prev, :, 512:1024],
                                  in_=osb[:, 512:1024])
            else:
                nc.vector.tensor_copy(osb[:], accT[:])
                nc.sync.dma_start(out=out_d[b_prev], in_=osb[:])

        def scores_round(b, emb_tiles, prev, emb_gen):
            """Emit scores+exp for batch b, interleaved with the PV of the
            previous batch and the (smeared) embed of the next batch, so
            the PE queue never has a long exp-gated stretch."""
            qT, kt, vaug, bcol = emb_tiles
            pt = pP.tile([128, NB, N], bf16, tag="pt")
            if prev is not None:
                accT = psAcc.tile([65, N], f32, tag="acc")
            else:
                accT = None
            nxt = None
            for mc in range(NB):
                S = psS.tile([128, N], f32, tag="S")
                lhs = kt[:, mc * 128:(mc + 1) * 128]
                nc.tensor.matmul(S[:, 0:512], lhs, qT[:, 0:512],
                                 start=True, stop=True)
                nc.tensor.matmul(S[:, 512:1024], lhs, qT[:, 512:1024],
                                 start=True, stop=True)
                nc.scalar.activation(out=pt[:, mc, :], in_=S[:],
                                     func=Act.Exp,
                                     bias=bcol[:, mc:mc + 1], scale=1.0)
                if prev is not None:
                    pv_chunk(prev, accT, mc)
                if emb_gen is not None and mc >= 1:
                    nxt = next(emb_gen, nxt) or nxt
            if prev is not None:
                pv_drain(prev[0], accT)
            return nxt, (b, pt, vaug)

        prev = None
        qt0 = fetch_qT(0)
        qt_next = fetch_qT(1)
        emb = None
        for emb in embed_steps(0, qt0):
            pass
        for b in range(BL):
            if b + 1 < BL:
                emb_gen = embed_steps(b + 1, qt_next)
                if b + 2 < BL:
                    qt_next = fetch_qT(b + 2)
            else:
                emb_gen = None
            nxt, prev = scores_round(b, emb, prev, emb_gen)
            emb = nxt
        accT = psAcc.tile([65, N], f32, tag="acc")
        for mc in range(NB):
            pv_chunk(prev, accT, mc)
        pv_drain(BL - 1, accT, split=True)


def _host_weights(Wq, Wk, Wv, Wq_s, Wk_s, bq_s):
    """Fold the tiny EquiLinear weights (float64 precompute)."""
    METRIC = np.array([1.0, -1.0, -1.0, -1.0], dtype=np.float64)
    G = Wq.astype(np.float64).T @ Wk.astype(np.float64)            # [16,16]
    BD = np.zeros((128, 128), dtype=np.float64)
    for k in range(4):
        # lhsT[(j',k), (j,k)] = SCALE * METRIC[k] * G[j, j']
        BD[k:64:4, k:64:4] = SCALE * METRIC[k] * G.T
    # lhsT[h, g] = SCALE * H[g, h],  H = Wq_s.T @ Wk_s
    BD[64:, 64:] = SCALE * (Wk_s.astype(np.float64).T @ Wq_s.astype(np.float64))
    E = np.exp(Wv.astype(np.float64))                              # [16,16]
    WvC2 = np.zeros((128, 65), dtype=np.float64)
    for k in range(4):
        # rhs[(j,k), (i,k)] = E[i, j]
        WvC2[k:64:4, k:64:4] = E.T
    WvC2[64:, 64] = SCALE * (Wk_s.astype(np.float64).T @ bq_s.astype(np.float64))
    return BD, WvC2


def _host_prep(vectors, scalars):
    """Lorentz-normalize and build q128^T = [vecs|scalars]^T per batch."""
    METRIC = np.array([1.0, -1.0, -1.0, -1.0], dtype=np.float32)
    v = np.asarray(vectors, dtype=np.float32)
    sq = v * v
    norm = (sq[..., 0] - sq[..., 1] - sq[..., 2] - sq[..., 3])[..., None]
    vecs = v / np.sqrt(np.clip(np.abs(norm), 1e-5, None))
    q128 = np.concatenate(
        [vecs.reshape(B, N, 64), np.asarray(scalars, dtype=np.float32)],
        axis=-1)                                      # [B, N, 128]
    qT = np.ascontiguousarray(q128.transpose(0, 2, 1))  # [B, 128, N]
    return qT, vecs.reshape(B, N, 64)


def _prepare_in_maps(vectors, scalars, Wq, Wq_s, bq_s, Wk, Wk_s, bk_s, Wv):
    import ml_dtypes
    BD, WvC2 = _host_weights(Wq, Wk, Wv, Wq_s, Wk_s, bq_s)
    qT, vecs = _host_prep(vectors, scalars)
    BD32 = np.ascontiguousarray(BD, dtype=np.float32)
    # Vaug (the tiny O(N) V-projection) + exp bias column, host-prepped:
    #   proj[b, m, 0:64] = V, proj[b, m, 64] = c2.s_m
    proj = np.einsum('bdm,dv->bmv', qT, WvC2.astype(np.float32),
                     optimize=True)                    # [B, N, 65]
    vaug = proj.reshape(B, NB, 128, 65).transpose(0, 2, 1, 3).copy()
    vaug[:, :, :, 64] = 1.0
    vaug16 = vaug.astype(ml_dtypes.bfloat16)           # [B, 128, NB, 65]
    bcol = (proj[:, :, 64] - CSHIFT).reshape(B, NB, 128) \
        .transpose(0, 2, 1).astype(np.float32)         # [B, 128, NB]
    in_maps = []
    for c in range(NCORES):
        sl = slice(c * BL, (c + 1) * BL)
        in_maps.append({
            "qT": np.ascontiguousarray(qT[sl]),
            "BD": BD32,
            "vaug": np.ascontiguousarray(vaug16[sl]),
            "bcol": np.ascontiguousarray(bcol[sl]),
        })
    return in_maps, (BD, WvC2, qT, vecs)


def _run(in_maps, **kw):
    from concourse.bass_utils import run_bass_kernel_spmd
    nc = _get_program()
    return run_bass_kernel_spmd(nc, in_maps, list(range(NCORES)), **kw)


def _get_program():
    if "nc" not in _CACHE:
        _CACHE["nc"] = _build_program()
    return _CACHE["nc"]


def _host_patch_row(b, q, BD, WvC2, qT, vecs):
    """Exact fp64 recompute of one (batch, query) output row."""
    q128 = qT[b].astype(np.float64)                   # [128, N]
    kq = BD @ q128[:, q]                              # [128]
    s_col = q128.T @ kq                               # S^T[m, q] = q_m^T BD q_q
    bias = q128[64:, :].T @ WvC2[64:, 64]             # c2 . s_m
    S = s_col + bias
    S -= S.max()
    P = np.exp(S)
    V = vecs[b].astype(np.float64) @ WvC2[0:64, 0:64]  # E-proj [N, 64]
    return (P @ V) / P.sum()


def kernel(vectors, scalars, Wq, Wq_s, bq_s, Wk, Wk_s, bk_s, Wv):
    args = [np.asarray(a, dtype=np.float32) for a in
            (vectors, scalars, Wq, Wq_s, bq_s, Wk, Wk_s, bk_s, Wv)]
    in_maps, host_ctx = _prepare_in_maps(*args)
    res = _run(in_maps)
    acc = np.concatenate([res.results[c]["out"] for c in range(NCORES)],
                         axis=0)                      # [B, 65, N]
    num = acc[:, 0:64, :]
    den = acc[:, 64, :]
    with np.errstate(divide="ignore", invalid="ignore", over="ignore"):
        out = (num / den[:, None, :]).transpose(0, 2, 1)   # [B, N, 64]
    # patch overflow-outlier rows exactly on host
    BD, WvC2, qT, vecs = host_ctx
    with np.errstate(over="ignore", invalid="ignore"):
        bad = ~np.isfinite(den) | (den <= 0) | \
            (np.log(np.maximum(den, 1e-30)) > FLAG_LOGDEN) | \
            ~np.isfinite(out).all(axis=2)
    for b, q in zip(*np.nonzero(bad)):
        out[b, q] = _host_patch_row(b, q, BD, WvC2, qT, vecs)
    return out.reshape(B, N, 16, 4).astype(np.float32)
